# revision 30
# baseline (speedup 1.0000x reference)
"""CrossAttentionFusion Trainium2 kernel.

Full-input contract: kernel(**inputs) takes the unsharded tensors and
returns the full [4, 128, 64, 64] output.

Sharding: 8 shards = (batch b in 0..3) x (image half in 0..1).  Each core
processes one image's context (all 4096 keys) and a 34-row query window
(32 output rows + halo rows for the trailing 3x3 conv), so there is no
cross-device communication.  Every core runs the same program; the host
slices inputs and reassembles outputs.

Per-core pipeline (all on one NeuronCore, Tile-scheduled):
  1. bilinear 2x upsample of context [256,32,32] -> [256,64,64]   (DVE,
     scale-folded: interp = a + b/3, the 0.5625 goes into Wk/Wv;
     H-pass chunked so k/v tiles unblock early)
  2. k/v/q 1x1 convs as fp32r matmuls + bias                      (PE+ACT)
  3. scores^T[m,n] = k^T q per 128-key chunk (fp32r), exp -> bf16 (PE+ACT)
  4. out^T[n, c|sum] = sum_m expT[m,n] * [v^T | 1] (bf16 matmuls); the
     appended ones column yields the softmax denominator for free  (PE)
  5. normalize by 1/sum, transpose back to [c, n] (bf16)          (DVE+PE)
  6. 3x3 conv as 9 shifted bf16 matmuls (gamma folded into Wp/bp),
     then one fused bias+residual op                              (PE+DVE)
"""

import os
import sys

for _p in ("/opt/trn_rl_repo", "/root/.axon_site/_ro/trn_rl_repo"):
    if os.path.isdir(_p) and _p not in sys.path:
        sys.path.insert(0, _p)

import ml_dtypes
import numpy as np

import concourse.bass as bass  # noqa: E402
import concourse.mybir as mybir  # noqa: E402
from concourse import bacc  # noqa: E402
from concourse.bass_utils import run_bass_kernel_spmd  # noqa: E402
from concourse.masks import make_identity  # noqa: E402
from concourse.tile import TileContext  # noqa: E402

B, C, H, W = 4, 128, 64, 64
Cc, Hc, Wc = 256, 32, 32
P = 128
N = H * W                 # keys per image
ROWS = 34                 # query-window rows (32 output + halo)
NQ = ROWS * W             # 2176 queries per core
M_CHUNKS = N // P         # 32 key chunks
# query blocks: multiples of 128 (PV chunking) and >=256 (fp32r speed)
ATT_BLOCKS = [(0, 512), (512, 512), (1024, 512), (1536, 384), (1920, 256)]
CONV_BLOCKS = [(0, 512), (512, 512), (1024, 512), (1536, 512), (2048, 128)]
F32 = mybir.dt.float32
F32R = mybir.dt.float32r
BF16 = mybir.dt.bfloat16
ALU = mybir.AluOpType
ACTF = mybir.ActivationFunctionType
IDENT = ACTF.Identity
THIRD = 1.0 / 3.0
FOUR3 = 4.0 / 3.0


def _build():
    nc = bacc.Bacc("TRN2", target_bir_lowering=False, debug=False)
    sr = nc.declare_dram_parameter("sr", [P, NQ], F32R, isOutput=False)
    ctx = nc.declare_dram_parameter("ctx", [P, 2, Hc, Wc], F32,
                                    isOutput=False)
    wq = nc.declare_dram_parameter("wq", [P, P], F32R, isOutput=False)
    wk = nc.declare_dram_parameter("wk", [P, 2, P], F32R, isOutput=False)
    wv = nc.declare_dram_parameter("wv", [P, 2, P], F32R, isOutput=False)
    wp = nc.declare_dram_parameter("wp", [P, 9, P], BF16, isOutput=False)
    # biases packed in one tensor: cols = bq | bk | bv | bp
    bia = nc.declare_dram_parameter("bias", [P, 4], F32, isOutput=False)
    outp = nc.declare_dram_parameter("out", [P, NQ], F32, isOutput=True)

    with TileContext(nc) as tc:
        with (
            tc.tile_pool(name="const", bufs=1) as cp,
        ):
            # data first (short critical path), weights on the gpsimd queue
            ctx_t = cp.tile([P, 2, Hc, Wc], F32)
            nc.sync.dma_start(ctx_t[:, 0], ctx[:, 0])
            nc.sync.dma_start(ctx_t[:, 1], ctx[:, 1])
            sr_t = cp.tile([P, NQ], F32R)
            nc.sync.dma_start(sr_t[:, 0:1024], sr[:, 0:1024])
            nc.sync.dma_start(sr_t[:, 1024:NQ], sr[:, 1024:NQ])
            wq_t = cp.tile([P, P], F32R)
            nc.gpsimd.dma_start(wq_t[:], wq[:])
            bia_t = cp.tile([P, 4], F32)
            nc.gpsimd.dma_start(bia_t[:], bia[:])
            bq_t, bk_t, bv_t, bp_t = (bia_t[:, i:i + 1] for i in range(4))
            wk_t = cp.tile([P, 2, P], F32R)
            nc.gpsimd.dma_start(wk_t[:], wk[:])
            wv_t = cp.tile([P, 2, P], F32R)
            nc.gpsimd.dma_start(wv_t[:], wv[:])
            wp_t = cp.tile([P, 9, P], BF16)
            nc.gpsimd.dma_start(wp_t[:], wp[:])

            k_t = cp.tile([P, N], F32R)
            q_t = cp.tile([P, NQ], F32R)
            ident_b = cp.tile([P, P], BF16)
            vTp = cp.tile([P, M_CHUNKS, P + 1], BF16)
            # zero-padded attention output for the 3x3 conv (bf16):
            # [ci, 36 rows, 66 cols]; window row r lives at row 1+r
            attn_c = cp.tile([P, ROWS + 2, W + 2], BF16)
            final = cp.tile([P, NQ], F32)

            # ---- phase 1: q conv, upsample context, k/v convs ----
            with (
                tc.tile_pool(name="ph1", bufs=1) as p1,
                tc.tile_pool(name="ph1ps", bufs=3, space="PSUM") as pps,
                tc.tile_pool(name="ph1tr", bufs=2, space="PSUM") as ptr,
            ):
                # q first: only needs sr + wq, keeps PE busy immediately
                for st, sz in ((0, 512), (512, 512), (1024, 512),
                               (1536, 512), (2048, 128)):
                    ps = pps.tile([P, 512], F32, tag="kv")
                    nc.tensor.matmul(ps[:, :sz], wq_t[:],
                                     sr_t[:, st:st + sz],
                                     start=True, stop=True)
                    nc.scalar.activation(q_t[:, st:st + sz], ps[:, :sz],
                                         IDENT, bias=bq_t)
                make_identity(nc, ident_b[:])

                # --- bilinear upsample, scale-folded (interp = a + b/3,
                # edges scaled by 4/3; the global 0.5625 is folded into
                # Wk/Wv on the host) ---
                ctxw = p1.tile([P, 2, Hc, W], F32)
                ctxu = p1.tile([P, 2, H, W], F32R)
                L = Hc
                for o in range(2):
                    src_o = ctx_t[:, o]
                    dw = ctxw[:, o].rearrange("p h (w t) -> p h w t", t=2)
                    nc.vector.tensor_scalar_mul(dw[:, :, 0, 0],
                                                src_o[:, :, 0], FOUR3)
                    nc.vector.tensor_scalar_mul(dw[:, :, L - 1, 1],
                                                src_o[:, :, L - 1], FOUR3)
                    # rows chunked so the H pass (and k/v tiles) can start
                    # before the whole W pass finishes; odd-parity work on
                    # the otherwise-idle GpSimd engine
                    for h0, h1 in ((0, 9), (9, 20), (20, Hc)):
                        nc.vector.scalar_tensor_tensor(
                            out=dw[:, h0:h1, 1:L, 0],
                            in0=src_o[:, h0:h1, 0:L - 1], scalar=THIRD,
                            in1=src_o[:, h0:h1, 1:L],
                            op0=ALU.mult, op1=ALU.add)
                        nc.vector.scalar_tensor_tensor(
                            out=dw[:, h0:h1, 0:L - 1, 1],
                            in0=src_o[:, h0:h1, 1:L], scalar=THIRD,
                            in1=src_o[:, h0:h1, 0:L - 1],
                            op0=ALU.mult, op1=ALU.add)
                dh = ctxu.rearrange("p o (h t) w -> p o h t w", t=2)
                ctxu_f = ctxu.rearrange("p o h w -> p o (h w)")
                v_sb = p1.tile([P, N], BF16)

                # H pass in 4 row-chunks of 8; after chunk hc the ctxu rows
                # 16hc..16hc+16 exist -> k/v tiles 2hc, 2hc+1 can run.
                for hc in range(4):
                    j0, j1 = 8 * hc, 8 * hc + 8
                    for o in range(2):
                        if hc == 0:
                            nc.vector.tensor_scalar_mul(
                                dh[:, o, 0, 0, :], ctxw[:, o, 0, :], FOUR3)
                        e0 = max(j0, 1)
                        nc.vector.scalar_tensor_tensor(
                            out=dh[:, o, e0:j1, 0, :],
                            in0=ctxw[:, o, e0 - 1:j1 - 1, :], scalar=THIRD,
                            in1=ctxw[:, o, e0:j1, :],
                            op0=ALU.mult, op1=ALU.add)
                        o1 = min(j1, L - 1)
                        nc.vector.scalar_tensor_tensor(
                            out=dh[:, o, j0:o1, 1, :],
                            in0=ctxw[:, o, j0 + 1:o1 + 1, :], scalar=THIRD,
                            in1=ctxw[:, o, j0:o1, :],
                            op0=ALU.mult, op1=ALU.add)
                        if hc == 3:
                            nc.vector.tensor_scalar_mul(
                                dh[:, o, L - 1, 1, :], ctxw[:, o, L - 1, :],
                                FOUR3)
                    for t in (2 * hc, 2 * hc + 1):
                        sl = slice(t * 512, (t + 1) * 512)
                        ps = pps.tile([P, 512], F32, tag="kv")
                        for cc in range(2):
                            nc.tensor.matmul(ps[:], wk_t[:, cc, :],
                                             ctxu_f[:, cc, sl],
                                             start=(cc == 0), stop=(cc == 1))
                        if t < 4:
                            nc.scalar.activation(k_t[:, sl], ps[:], IDENT,
                                                 bias=bk_t)
                        else:
                            nc.vector.tensor_scalar_add(k_t[:, sl], ps[:],
                                                        bk_t)
                        ps2 = pps.tile([P, 512], F32, tag="kv")
                        for cc in range(2):
                            nc.tensor.matmul(ps2[:], wv_t[:, cc, :],
                                             ctxu_f[:, cc, sl],
                                             start=(cc == 0), stop=(cc == 1))
                        if t < 4:
                            nc.scalar.activation(v_sb[:, sl], ps2[:], IDENT,
                                                 bias=bv_t)
                        else:
                            nc.vector.tensor_scalar_add(v_sb[:, sl], ps2[:],
                                                        bv_t)
                        # v^T for the PV matmuls (bf16, ones col appended)
                        for j in range(4 * t, 4 * t + 4):
                            tp = ptr.tile([P, P], BF16, tag="vtr")
                            nc.tensor.transpose(
                                tp[:], v_sb[:, j * P:(j + 1) * P],
                                ident_b[:])
                            nc.vector.tensor_copy(out=vTp[:, j, 0:P],
                                                  in_=tp[:])

            # ---- phase 2+3: attention with interleaved conv ----
            # Emission order drives Tile priorities: QK pairs of block nb
            # interleave with PV chunks of block nb-1 (PV's long bf16
            # streams hide the fp32r QK weight loads), and each 3x3-conv
            # block is emitted as soon as the attn_c rows it reads exist.
            with (
                tc.tile_pool(name="att", bufs=2) as ab,
                tc.tile_pool(name="attsm", bufs=3) as asml,
                tc.tile_pool(name="qkps", bufs=2, space="PSUM") as qkps,
                tc.tile_pool(name="pvps", bufs=3, space="PSUM") as pvps,
                tc.tile_pool(name="cvps", bufs=1, space="PSUM") as cvps,
            ):
                nc.gpsimd.memset(vTp[:, :, P:P + 1], 1.0)
                nc.gpsimd.memset(attn_c[:], 0.0)
                exp_tiles = {}

                def emit_qk_pair(nb, jj):
                    # one 2-bank psum tile holds g key-chunks; placement
                    # stride keeps every matmul write inside one 2KB bank
                    nstart, bsz = ATT_BLOCKS[nb]
                    g, stride = (2, 512) if bsz >= 384 else (4, bsz)
                    if jj == 0:
                        exp_tiles[nb] = ab.tile([P, M_CHUNKS, 512], BF16,
                                                tag="expT", name="expT")
                    expT = exp_tiles[nb]
                    ps = qkps.tile([P, 2, 512], F32, tag="qk")
                    psf = ps.rearrange("p a b -> p (a b)")
                    for h2 in range(g):
                        j = g * jj + h2
                        nc.tensor.matmul(
                            psf[:, h2 * stride:h2 * stride + bsz],
                            k_t[:, j * P:(j + 1) * P],
                            q_t[:, nstart:nstart + bsz],
                            start=True, stop=True)
                    nc.scalar.activation(
                        expT[:, g * jj:g * jj + g, :bsz],
                        psf.rearrange("p (a b) -> p a b", b=stride)
                        [:, :g, :bsz], ACTF.Exp)

                def emit_pv_chunk(nb, ci):
                    nstart, bsz = ATT_BLOCKS[nb]
                    expT = exp_tiles[nb]
                    chunk = nstart // P + ci
                    po = pvps.tile([P, P + 1], F32, tag="pv")
                    for j in range(M_CHUNKS):
                        nc.tensor.matmul(
                            po[:], expT[:, j, ci * P:(ci + 1) * P],
                            vTp[:, j, :],
                            start=(j == 0), stop=(j == M_CHUNKS - 1))
                    rec = asml.tile([P, 1], F32, tag="rec")
                    nc.vector.reciprocal(rec[:], po[:, P:P + 1])
                    attn_T = asml.tile([P, P], BF16, tag="attnT")
                    nc.vector.tensor_scalar_mul(attn_T[:], po[:, 0:P],
                                                rec[:])
                    # transpose target shares the pv slot tag (same bytes)
                    tp_raw = pvps.tile([P, P + 1], F32, tag="pv",
                                       name="tp_raw")
                    tp = tp_raw.bitcast(BF16)[:, 0:P]
                    nc.tensor.transpose(tp, attn_T[:], ident_b[:])
                    r = chunk * 2  # window row of this chunk
                    nc.vector.tensor_copy(
                        out=attn_c[:, 1 + r:3 + r, 1:W + 1],
                        in_=tp.rearrange("p (r w) -> p r w", w=W))

                def emit_conv_block(cb):
                    st, sz = CONV_BLOCKS[cb]
                    row0 = st // W
                    nrows = sz // W
                    ps = cvps.tile([P, 512], F32, tag="cv")
                    idx = 0
                    for ky in range(3):
                        for kx in range(3):
                            rhs = attn_c[:, row0 + ky:row0 + ky + nrows,
                                         kx:kx + W]
                            nc.tensor.matmul(ps[:, :sz],
                                             wp_t[:, ky * 3 + kx, :], rhs,
                                             start=(idx == 0),
                                             stop=(idx == 8))
                            idx += 1
                    # final = conv + gamma*bp + sr   (gamma in wp/bp)
                    nc.vector.scalar_tensor_tensor(
                        out=final[:, st:st + sz],
                        in0=ps[:, :sz], scalar=bp_t,
                        in1=sr_t.bitcast(F32)[:, st:st + sz],
                        op0=ALU.add, op1=ALU.add)
                    nc.sync.dma_start(outp[:, st:st + sz],
                                      final[:, st:st + sz])

                NB = len(ATT_BLOCKS)
                chunks_of = [bsz // P for _, bsz in ATT_BLOCKS]
                done_chunks = 0
                next_conv = 0

                def after_chunk():
                    # conv block cb reads attn_c rows up to 8*cb+9, i.e.
                    # chunks up to 4*cb+4 (chunk = 2 rows)
                    nonlocal next_conv
                    while (next_conv < len(CONV_BLOCKS)
                           and done_chunks >= min(4 * next_conv + 5, 17)):
                        emit_conv_block(next_conv)
                        next_conv += 1

                for nb in range(NB):
                    prev = nb - 1
                    nprev = chunks_of[prev] if prev >= 0 else 0
                    npair = M_CHUNKS // (2 if ATT_BLOCKS[nb][1] >= 384
                                         else 4)
                    step = (npair + 3) // 4
                    for grp in range(4):
                        for jj in range(step * grp,
                                        min(step * (grp + 1), npair)):
                            emit_qk_pair(nb, jj)
                        if prev >= 0 and grp < nprev:
                            emit_pv_chunk(prev, grp)
                            done_chunks += 1
                            after_chunk()
                for ci in range(chunks_of[NB - 1]):
                    emit_pv_chunk(NB - 1, ci)
                    done_chunks += 1
                    after_chunk()
                while next_conv < len(CONV_BLOCKS):
                    emit_conv_block(next_conv)
                    next_conv += 1

    nc.compile()
    return nc


_CACHE = {}


def _get_program():
    if "nc" not in _CACHE:
        _CACHE["nc"] = _build()
    return _CACHE["nc"]


UPS = 0.5625  # (3/4)^2 upsample scale folded into Wk/Wv


def _prep_inputs(sr_feat, context_feat, Wq, bq, Wk, bk, Wv, bv, Wp, bp,
                 gamma):
    f32 = np.float32
    bf16 = ml_dtypes.bfloat16
    sr_feat = np.asarray(sr_feat, f32)
    context_feat = np.asarray(context_feat, f32)
    g = np.asarray(gamma, f32)[0]
    shared = {
        "wq": np.ascontiguousarray(np.asarray(Wq, f32)[:, :, 0, 0].T),
        "wk": np.ascontiguousarray(
            (np.asarray(Wk, f32) * UPS)[:, :, 0, 0].T.reshape(2, P, P)
            .transpose(1, 0, 2)),
        "wv": np.ascontiguousarray(
            (np.asarray(Wv, f32) * UPS)[:, :, 0, 0].T.reshape(2, P, P)
            .transpose(1, 0, 2)),
        "wp": np.ascontiguousarray(
            (np.asarray(Wp, f32) * g).transpose(2, 3, 1, 0).reshape(9, P, P)
            .transpose(1, 0, 2)).astype(bf16),
        "bias": np.ascontiguousarray(np.stack(
            [np.asarray(bq, f32), np.asarray(bk, f32),
             np.asarray(bv, f32), np.asarray(bp, f32) * g], axis=1)),
    }
    in_maps = []
    for s in range(8):
        b, half = divmod(s, 2)
        r0 = 0 if half == 0 else H - ROWS
        m = dict(shared)
        m["sr"] = np.ascontiguousarray(
            sr_feat[b, :, r0:r0 + ROWS, :]).reshape(P, NQ)
        m["ctx"] = np.ascontiguousarray(
            context_feat[b].reshape(2, P, Hc, Wc).transpose(1, 0, 2, 3))
        in_maps.append(m)
    return in_maps


def _assemble(results):
    out = np.empty((B, C, H, W), np.float32)
    for s in range(8):
        b, half = divmod(s, 2)
        off = 0 if half == 0 else 2  # output rows within the 34-row window
        y = results[s]["out"].reshape(P, ROWS, W)
        out[b, :, half * 32:(half + 1) * 32, :] = y[:, off:off + 32, :]
    return out


def kernel(**inputs):
    nc = _get_program()
    in_maps = _prep_inputs(**inputs)
    res = run_bass_kernel_spmd(nc, in_maps, list(range(8)))
    return _assemble(res.results)


def kernel_traced(**inputs):
    """Like kernel() but also returns the hardware exec time in ns."""
    nc = _get_program()
    in_maps = _prep_inputs(**inputs)
    res = run_bass_kernel_spmd(nc, in_maps, list(range(8)), trace=True)
    return _assemble(res.results), res


# revision 31
# speedup vs baseline: 1.0390x; 1.0390x over previous
"""CrossAttentionFusion Trainium2 kernel.

Full-input contract: kernel(**inputs) takes the unsharded tensors and
returns the full [4, 128, 64, 64] output.

Sharding: 8 shards = (batch b in 0..3) x (image half in 0..1).  Each core
processes one image's context (all 4096 keys) and a 34-row query window
(32 output rows + halo rows for the trailing 3x3 conv), so there is no
cross-device communication.  Every core runs the same program; the host
slices inputs and reassembles outputs.

Per-core pipeline (all on one NeuronCore, Tile-scheduled):
  1. bilinear 2x upsample of context [256,32,32] -> [256,64,64]   (DVE,
     scale-folded: interp = a + b/3, the 0.5625 goes into Wk/Wv;
     H-pass chunked so k/v tiles unblock early)
  2. k/v/q 1x1 convs as fp32r matmuls + bias                      (PE+ACT)
  3. scores^T[m,n] = k^T q per 128-key chunk (fp32r), exp -> bf16 (PE+ACT)
  4. out^T[n, c|sum] = sum_m expT[m,n] * [v^T | 1] (bf16 matmuls); the
     appended ones column yields the softmax denominator for free  (PE)
  5. normalize by 1/sum, transpose back to [c, n] (bf16)          (DVE+PE)
  6. 3x3 conv as 9 shifted bf16 matmuls (gamma folded into Wp/bp),
     then one fused bias+residual op                              (PE+DVE)
"""

import os
import sys

for _p in ("/opt/trn_rl_repo", "/root/.axon_site/_ro/trn_rl_repo"):
    if os.path.isdir(_p) and _p not in sys.path:
        sys.path.insert(0, _p)

import ml_dtypes
import numpy as np

import concourse.bass as bass  # noqa: E402
import concourse.mybir as mybir  # noqa: E402
from concourse import bacc  # noqa: E402
from concourse.bass_utils import run_bass_kernel_spmd  # noqa: E402
from concourse.masks import make_identity  # noqa: E402
from concourse.tile import TileContext  # noqa: E402

B, C, H, W = 4, 128, 64, 64
Cc, Hc, Wc = 256, 32, 32
P = 128
N = H * W                 # keys per image
ROWS = 34                 # query-window rows (32 output + halo)
NQ = ROWS * W             # 2176 queries per core
M_CHUNKS = N // P         # 32 key chunks
# query blocks: multiples of 128 (PV chunking) and >=256 (fp32r speed)
ATT_BLOCKS = [(0, 512), (512, 512), (1024, 512), (1536, 384), (1920, 256)]
CONV_BLOCKS = [(0, 512), (512, 512), (1024, 512), (1536, 512), (2048, 128)]
F32 = mybir.dt.float32
F32R = mybir.dt.float32r
BF16 = mybir.dt.bfloat16
ALU = mybir.AluOpType
ACTF = mybir.ActivationFunctionType
IDENT = ACTF.Identity
THIRD = 1.0 / 3.0
FOUR3 = 4.0 / 3.0


def _build():
    nc = bacc.Bacc("TRN2", target_bir_lowering=False, debug=False)
    sr = nc.declare_dram_parameter("sr", [P, NQ], F32R, isOutput=False)
    ctx = nc.declare_dram_parameter("ctx", [P, 2, Hc, Wc], F32,
                                    isOutput=False)
    wq = nc.declare_dram_parameter("wq", [P, P], F32R, isOutput=False)
    wk = nc.declare_dram_parameter("wk", [P, 2, P], F32R, isOutput=False)
    wv = nc.declare_dram_parameter("wv", [P, 2, P], F32R, isOutput=False)
    wp = nc.declare_dram_parameter("wp", [P, 9, P], BF16, isOutput=False)
    # biases packed in one tensor: cols = bq | bk | bv | bp
    bia = nc.declare_dram_parameter("bias", [P, 4], F32, isOutput=False)
    outp = nc.declare_dram_parameter("out", [P, NQ], F32, isOutput=True)

    with TileContext(nc) as tc:
        with (
            tc.tile_pool(name="const", bufs=1) as cp,
        ):
            # data first (short critical path), weights on the gpsimd queue
            ctx_t = cp.tile([P, 2, Hc, Wc], F32)
            nc.sync.dma_start(ctx_t[:, 0], ctx[:, 0])
            nc.sync.dma_start(ctx_t[:, 1], ctx[:, 1])
            sr_t = cp.tile([P, NQ], F32R)
            nc.sync.dma_start(sr_t[:, 0:1024], sr[:, 0:1024])
            nc.sync.dma_start(sr_t[:, 1024:NQ], sr[:, 1024:NQ])
            wq_t = cp.tile([P, P], F32R)
            nc.gpsimd.dma_start(wq_t[:], wq[:])
            bia_t = cp.tile([P, 4], F32)
            nc.gpsimd.dma_start(bia_t[:], bia[:])
            bq_t, bk_t, bv_t, bp_t = (bia_t[:, i:i + 1] for i in range(4))
            wk_t = cp.tile([P, 2, P], F32R)
            nc.gpsimd.dma_start(wk_t[:], wk[:])
            wv_t = cp.tile([P, 2, P], F32R)
            nc.gpsimd.dma_start(wv_t[:], wv[:])
            wp_t = cp.tile([P, 9, P], BF16)
            nc.gpsimd.dma_start(wp_t[:], wp[:])

            k_t = cp.tile([P, N], F32R)
            q_t = cp.tile([P, NQ], F32R)
            ident_b = cp.tile([P, P], BF16)
            vTp = cp.tile([P, M_CHUNKS, P + 1], BF16)
            # zero-padded attention output for the 3x3 conv (bf16):
            # [ci, 36 rows, 66 cols]; window row r lives at row 1+r
            attn_c = cp.tile([P, ROWS + 2, W + 2], BF16)
            final = cp.tile([P, NQ], F32)

            # ---- phase 1: q conv, upsample context, k/v convs ----
            with (
                tc.tile_pool(name="ph1", bufs=1) as p1,
                tc.tile_pool(name="ph1ps", bufs=3, space="PSUM") as pps,
                tc.tile_pool(name="ph1tr", bufs=2, space="PSUM") as ptr,
            ):
                # q first: only needs sr + wq, keeps PE busy immediately
                for st, sz in ((0, 512), (512, 512), (1024, 512),
                               (1536, 512), (2048, 128)):
                    ps = pps.tile([P, 512], F32, tag="kv")
                    nc.tensor.matmul(ps[:, :sz], wq_t[:],
                                     sr_t[:, st:st + sz],
                                     start=True, stop=True)
                    nc.scalar.activation(q_t[:, st:st + sz], ps[:, :sz],
                                         IDENT, bias=bq_t)
                make_identity(nc, ident_b[:])

                # --- bilinear upsample, scale-folded (interp = a + b/3,
                # edges scaled by 4/3; the global 0.5625 is folded into
                # Wk/Wv on the host) ---
                ctxw = p1.tile([P, 2, Hc, W], F32)
                ctxu = p1.tile([P, 2, H, W], F32R)
                L = Hc
                for o in range(2):
                    src_o = ctx_t[:, o]
                    dw = ctxw[:, o].rearrange("p h (w t) -> p h w t", t=2)
                    nc.vector.tensor_scalar_mul(dw[:, :, 0, 0],
                                                src_o[:, :, 0], FOUR3)
                    nc.vector.tensor_scalar_mul(dw[:, :, L - 1, 1],
                                                src_o[:, :, L - 1], FOUR3)
                    # rows chunked so the H pass (and k/v tiles) can start
                    # before the whole W pass finishes; odd-parity work on
                    # the otherwise-idle GpSimd engine
                    for h0, h1 in ((0, 9), (9, 20), (20, Hc)):
                        nc.vector.scalar_tensor_tensor(
                            out=dw[:, h0:h1, 1:L, 0],
                            in0=src_o[:, h0:h1, 0:L - 1], scalar=THIRD,
                            in1=src_o[:, h0:h1, 1:L],
                            op0=ALU.mult, op1=ALU.add)
                        nc.vector.scalar_tensor_tensor(
                            out=dw[:, h0:h1, 0:L - 1, 1],
                            in0=src_o[:, h0:h1, 1:L], scalar=THIRD,
                            in1=src_o[:, h0:h1, 0:L - 1],
                            op0=ALU.mult, op1=ALU.add)
                dh = ctxu.rearrange("p o (h t) w -> p o h t w", t=2)
                ctxu_f = ctxu.rearrange("p o h w -> p o (h w)")
                v_sb = p1.tile([P, N], BF16)

                # H pass in 4 row-chunks of 8; after chunk hc the ctxu rows
                # 16hc..16hc+16 exist -> k/v tiles 2hc, 2hc+1 can run.
                for hc in range(4):
                    j0, j1 = 8 * hc, 8 * hc + 8
                    for o in range(2):
                        if hc == 0:
                            nc.vector.tensor_scalar_mul(
                                dh[:, o, 0, 0, :], ctxw[:, o, 0, :], FOUR3)
                        e0 = max(j0, 1)
                        nc.vector.scalar_tensor_tensor(
                            out=dh[:, o, e0:j1, 0, :],
                            in0=ctxw[:, o, e0 - 1:j1 - 1, :], scalar=THIRD,
                            in1=ctxw[:, o, e0:j1, :],
                            op0=ALU.mult, op1=ALU.add)
                        o1 = min(j1, L - 1)
                        nc.vector.scalar_tensor_tensor(
                            out=dh[:, o, j0:o1, 1, :],
                            in0=ctxw[:, o, j0 + 1:o1 + 1, :], scalar=THIRD,
                            in1=ctxw[:, o, j0:o1, :],
                            op0=ALU.mult, op1=ALU.add)
                        if hc == 3:
                            nc.vector.tensor_scalar_mul(
                                dh[:, o, L - 1, 1, :], ctxw[:, o, L - 1, :],
                                FOUR3)
                    for t in (2 * hc, 2 * hc + 1):
                        sl = slice(t * 512, (t + 1) * 512)
                        ps = pps.tile([P, 512], F32, tag="kv")
                        for cc in range(2):
                            nc.tensor.matmul(ps[:], wk_t[:, cc, :],
                                             ctxu_f[:, cc, sl],
                                             start=(cc == 0), stop=(cc == 1))
                        nc.scalar.activation(k_t[:, sl], ps[:], IDENT,
                                             bias=bk_t)
                        ps2 = pps.tile([P, 512], F32, tag="kv")
                        for cc in range(2):
                            nc.tensor.matmul(ps2[:], wv_t[:, cc, :],
                                             ctxu_f[:, cc, sl],
                                             start=(cc == 0), stop=(cc == 1))
                        nc.scalar.activation(v_sb[:, sl], ps2[:], IDENT,
                                             bias=bv_t)
                        # v^T for the PV matmuls (bf16, ones col appended)
                        for j in range(4 * t, 4 * t + 4):
                            tp = ptr.tile([P, P], BF16, tag="vtr")
                            nc.tensor.transpose(
                                tp[:], v_sb[:, j * P:(j + 1) * P],
                                ident_b[:])
                            nc.vector.tensor_copy(out=vTp[:, j, 0:P],
                                                  in_=tp[:])

            # ---- phase 2+3: attention with interleaved conv ----
            # Emission order drives Tile priorities: QK pairs of block nb
            # interleave with PV chunks of block nb-1 (PV's long bf16
            # streams hide the fp32r QK weight loads), and each 3x3-conv
            # block is emitted as soon as the attn_c rows it reads exist.
            with (
                tc.tile_pool(name="att", bufs=2) as ab,
                tc.tile_pool(name="attsm", bufs=3) as asml,
                tc.tile_pool(name="qkps", bufs=2, space="PSUM") as qkps,
                tc.tile_pool(name="pvps", bufs=3, space="PSUM") as pvps,
                tc.tile_pool(name="cvps", bufs=1, space="PSUM") as cvps,
            ):
                nc.gpsimd.memset(vTp[:, :, P:P + 1], 1.0)
                nc.gpsimd.memset(attn_c[:], 0.0)
                exp_tiles = {}

                def emit_qk_pair(nb, jj):
                    nstart, bsz = ATT_BLOCKS[nb]
                    if jj == 0:
                        exp_tiles[nb] = ab.tile([P, M_CHUNKS, 512], BF16,
                                                tag="expT", name="expT")
                    expT = exp_tiles[nb]
                    ps = qkps.tile([P, 2, 512], F32, tag="qk")
                    for h2 in range(2):
                        j = 2 * jj + h2
                        nc.tensor.matmul(ps[:, h2, :bsz],
                                         k_t[:, j * P:(j + 1) * P],
                                         q_t[:, nstart:nstart + bsz],
                                         start=True, stop=True)
                    nc.scalar.activation(expT[:, 2 * jj:2 * jj + 2, :bsz],
                                         ps[:, :, :bsz], ACTF.Exp)

                def emit_pv_chunk(nb, ci):
                    nstart, bsz = ATT_BLOCKS[nb]
                    expT = exp_tiles[nb]
                    chunk = nstart // P + ci
                    po = pvps.tile([P, P + 1], F32, tag="pv")
                    for j in range(M_CHUNKS):
                        nc.tensor.matmul(
                            po[:], expT[:, j, ci * P:(ci + 1) * P],
                            vTp[:, j, :],
                            start=(j == 0), stop=(j == M_CHUNKS - 1))
                    rec = asml.tile([P, 1], F32, tag="rec")
                    nc.vector.reciprocal(rec[:], po[:, P:P + 1])
                    attn_T = asml.tile([P, P], BF16, tag="attnT")
                    nc.vector.tensor_scalar_mul(attn_T[:], po[:, 0:P],
                                                rec[:])
                    # transpose target shares the pv slot tag (same bytes)
                    tp_raw = pvps.tile([P, P + 1], F32, tag="pv",
                                       name="tp_raw")
                    tp = tp_raw.bitcast(BF16)[:, 0:P]
                    nc.tensor.transpose(tp, attn_T[:], ident_b[:])
                    r = chunk * 2  # window row of this chunk
                    nc.vector.tensor_copy(
                        out=attn_c[:, 1 + r:3 + r, 1:W + 1],
                        in_=tp.rearrange("p (r w) -> p r w", w=W))

                def emit_conv_block(cb):
                    st, sz = CONV_BLOCKS[cb]
                    row0 = st // W
                    nrows = sz // W
                    ps = cvps.tile([P, 512], F32, tag="cv")
                    idx = 0
                    for ky in range(3):
                        for kx in range(3):
                            rhs = attn_c[:, row0 + ky:row0 + ky + nrows,
                                         kx:kx + W]
                            nc.tensor.matmul(ps[:, :sz],
                                             wp_t[:, ky * 3 + kx, :], rhs,
                                             start=(idx == 0),
                                             stop=(idx == 8))
                            idx += 1
                    # final = conv + gamma*bp + sr   (gamma in wp/bp)
                    nc.vector.scalar_tensor_tensor(
                        out=final[:, st:st + sz],
                        in0=ps[:, :sz], scalar=bp_t,
                        in1=sr_t.bitcast(F32)[:, st:st + sz],
                        op0=ALU.add, op1=ALU.add)
                    nc.sync.dma_start(outp[:, st:st + sz],
                                      final[:, st:st + sz])

                NB = len(ATT_BLOCKS)
                chunks_of = [bsz // P for _, bsz in ATT_BLOCKS]
                done_chunks = 0
                next_conv = 0

                def after_chunk():
                    # conv block cb reads attn_c rows up to 8*cb+9, i.e.
                    # chunks up to 4*cb+4 (chunk = 2 rows)
                    nonlocal next_conv
                    while (next_conv < len(CONV_BLOCKS)
                           and done_chunks >= min(4 * next_conv + 5, 17)):
                        emit_conv_block(next_conv)
                        next_conv += 1

                for nb in range(NB):
                    prev = nb - 1
                    nprev = chunks_of[prev] if prev >= 0 else 0
                    for g in range(4):
                        for jj in range(4 * g, 4 * g + 4):
                            emit_qk_pair(nb, jj)
                        if prev >= 0 and g < nprev:
                            emit_pv_chunk(prev, g)
                            done_chunks += 1
                            after_chunk()
                for ci in range(chunks_of[NB - 1]):
                    emit_pv_chunk(NB - 1, ci)
                    done_chunks += 1
                    after_chunk()
                while next_conv < len(CONV_BLOCKS):
                    emit_conv_block(next_conv)
                    next_conv += 1

    nc.compile()
    return nc


_CACHE = {}


def _get_program():
    if "nc" not in _CACHE:
        _CACHE["nc"] = _build()
    return _CACHE["nc"]


UPS = 0.5625  # (3/4)^2 upsample scale folded into Wk/Wv


def _prep_inputs(sr_feat, context_feat, Wq, bq, Wk, bk, Wv, bv, Wp, bp,
                 gamma):
    f32 = np.float32
    bf16 = ml_dtypes.bfloat16
    sr_feat = np.asarray(sr_feat, f32)
    context_feat = np.asarray(context_feat, f32)
    g = np.asarray(gamma, f32)[0]
    shared = {
        "wq": np.ascontiguousarray(np.asarray(Wq, f32)[:, :, 0, 0].T),
        "wk": np.ascontiguousarray(
            (np.asarray(Wk, f32) * UPS)[:, :, 0, 0].T.reshape(2, P, P)
            .transpose(1, 0, 2)),
        "wv": np.ascontiguousarray(
            (np.asarray(Wv, f32) * UPS)[:, :, 0, 0].T.reshape(2, P, P)
            .transpose(1, 0, 2)),
        "wp": np.ascontiguousarray(
            (np.asarray(Wp, f32) * g).transpose(2, 3, 1, 0).reshape(9, P, P)
            .transpose(1, 0, 2)).astype(bf16),
        "bias": np.ascontiguousarray(np.stack(
            [np.asarray(bq, f32), np.asarray(bk, f32),
             np.asarray(bv, f32), np.asarray(bp, f32) * g], axis=1)),
    }
    in_maps = []
    for s in range(8):
        b, half = divmod(s, 2)
        r0 = 0 if half == 0 else H - ROWS
        m = dict(shared)
        m["sr"] = np.ascontiguousarray(
            sr_feat[b, :, r0:r0 + ROWS, :]).reshape(P, NQ)
        m["ctx"] = np.ascontiguousarray(
            context_feat[b].reshape(2, P, Hc, Wc).transpose(1, 0, 2, 3))
        in_maps.append(m)
    return in_maps


def _assemble(results):
    out = np.empty((B, C, H, W), np.float32)
    for s in range(8):
        b, half = divmod(s, 2)
        off = 0 if half == 0 else 2  # output rows within the 34-row window
        y = results[s]["out"].reshape(P, ROWS, W)
        out[b, :, half * 32:(half + 1) * 32, :] = y[:, off:off + 32, :]
    return out


def kernel(**inputs):
    nc = _get_program()
    in_maps = _prep_inputs(**inputs)
    res = run_bass_kernel_spmd(nc, in_maps, list(range(8)))
    return _assemble(res.results)


def kernel_traced(**inputs):
    """Like kernel() but also returns the hardware exec time in ns."""
    nc = _get_program()
    in_maps = _prep_inputs(**inputs)
    res = run_bass_kernel_spmd(nc, in_maps, list(range(8)), trace=True)
    return _assemble(res.results), res
